# revision 11
# baseline (speedup 1.0000x reference)
"""Trainium2 Bass kernel for nn_AttentionBiasHead.

Per-sample attention with a post-softmax additive bias produced by an MLP whose
output Linear is huge (128 x 262144).  Strategy (8 NeuronCores):

- Data-parallel over batch: core i owns samples [4i, 4i+4).
- The bias-MLP output Linear (Wb2) is column-sharded: core i loads the
  columns for query rows l1 in [64i, 64(i+1)) and computes those bias rows
  for ALL 32 samples; an fp16 AllToAll then delivers to each core the full
  bias for its own 4 samples.  The AllToAll runs on the collective engines
  and overlaps with the attention compute.
- Inputs are pre-transposed/cast on the host (layout prep): q/k/v arrive as
  [din, l] fp16 so every matmul contracts over the partition dim with N=512.
"""

import numpy as np

N_CORES = 8
B, L, DIN, DQ, DS, DMLP = 32, 512, 512, 128, 256, 128
BPC = B // N_CORES          # samples per core = 4
NSH = L * L // N_CORES      # bias-shard columns per core = 32768
NT = NSH // 512             # Wb2 GEMM tiles = 64
NG = 8                      # GEMM tiles per DMA group
KT = DIN // 128             # contraction tiles for projections = 4
NC1 = L // 128              # l1 chunks per sample = 4
SCALE = 1.0 / float(np.sqrt(DQ))

_cache = {}


def _build():
    if "nc" in _cache:
        return _cache["nc"]

    from contextlib import ExitStack

    import concourse.mybir as mybir
    import concourse.tile as tile
    from concourse import bacc
    from concourse.bass import ts
    from concourse.masks import make_identity

    dt = mybir.dt
    f32, f16, u8 = dt.float32, dt.float16, dt.uint8

    nc = bacc.Bacc("TRN2", target_bir_lowering=False, debug=False,
                   num_devices=N_CORES)

    # ---- per-core external tensors -------------------------------------
    qT_d = nc.dram_tensor("qT", [BPC, DIN, L], f16, kind="ExternalInput").ap()
    kT_d = nc.dram_tensor("kT", [BPC, DIN, L], f16, kind="ExternalInput").ap()
    vT_d = nc.dram_tensor("vT", [BPC, DIN, L], f16, kind="ExternalInput").ap()
    mk_d = nc.dram_tensor("mk", [BPC, L, L], u8, kind="ExternalInput").ap()
    sfT_d = nc.dram_tensor("sfT", [DS, B], f32, kind="ExternalInput").ap()
    wqkv_d = nc.dram_tensor("wqkv", [DIN, 3, DQ], f16, kind="ExternalInput").ap()
    bias4_d = nc.dram_tensor("bias4", [128, 4], f32, kind="ExternalInput").ap()
    Wb1_d = nc.dram_tensor("Wb1", [DS, DMLP], f32, kind="ExternalInput").ap()
    Wb2s_d = nc.dram_tensor("Wb2s", [DMLP, NSH], f16, kind="ExternalInput").ap()
    bb2r_d = nc.dram_tensor("bb2r", [L, L], f16, kind="ExternalInput").ap()
    out_d = nc.dram_tensor("out", [BPC, L, DQ], f32, kind="ExternalOutput").ap()

    with tile.TileContext(nc) as tc, ExitStack() as ctx:
        consts = ctx.enter_context(tc.tile_pool(name="consts", bufs=1))
        dram = ctx.enter_context(tc.tile_pool(name="dram", bufs=1, space="DRAM"))

        # ---- hoisted input loads (ACT HW ring / gpsimd, issue early) ---
        inp = ctx.enter_context(tc.tile_pool(name="inp", bufs=BPC))
        mskp = ctx.enter_context(tc.tile_pool(name="mskp", bufs=BPC))
        qTin, kTin, vTin, mtile = {}, {}, {}, {}
        for s in range(BPC):
            qTin[s] = inp.tile([128, KT, L], f16, tag="qTin", name=f"qTin{s}")
            nc.gpsimd.dma_start(qTin[s][:],
                                qT_d[s].rearrange("(kt p) l -> p kt l", p=128))
            kTin[s] = inp.tile([128, KT, L], f16, tag="kTin", name=f"kTin{s}")
            nc.gpsimd.dma_start(kTin[s][:],
                                kT_d[s].rearrange("(kt p) l -> p kt l", p=128))
            vTin[s] = inp.tile([128, KT, L], f16, tag="vTin", name=f"vTin{s}")
            nc.gpsimd.dma_start(vTin[s][:],
                                vT_d[s].rearrange("(kt p) l -> p kt l", p=128))
            mtile[s] = mskp.tile([128, NC1, L], u8, tag="mt", name=f"mt{s}")
            nc.gpsimd.dma_start(mtile[s][:],
                                mk_d[s].rearrange("(c p) l -> p c l", p=128))

        bb2_sb = consts.tile([128, NC1, 512], f16)
        nc.gpsimd.dma_start(bb2_sb[:], bb2r_d.rearrange("(c p) l -> p c l", p=128))
        wqkv_sb = consts.tile([128, KT, 3, DQ], f16)
        nc.gpsimd.dma_start(wqkv_sb[:],
                            wqkv_d.rearrange("(kt p) w d -> p kt w d", p=128))

        ident16 = consts.tile([128, 128], f16)
        make_identity(nc, ident16)
        ident32 = consts.tile([128, 128], f32)
        make_identity(nc, ident32)
        c9 = consts.tile([128, 1], f32)
        nc.vector.memset(c9, 1e-9)

        # ---- sync ring: bias-pipeline tensors first --------------------
        sfT_sb = consts.tile([128, DS // 128, B], f32)
        nc.sync.dma_start(sfT_sb[:], sfT_d.rearrange("(kt p) b -> p kt b", p=128))
        Wb1_sb = consts.tile([128, DS // 128, DMLP], f32)
        nc.sync.dma_start(Wb1_sb[:], Wb1_d.rearrange("(kt p) d -> p kt d", p=128))
        bias4_sb = consts.tile([128, 4], f32)
        nc.sync.dma_start(bias4_sb[:], bias4_d[:])

        # ---- phase A: H^T = relu(Wb1^T @ sf^T + bb1)  [128, 32] --------
        with tc.tile_pool(name="htps", bufs=1, space="PSUM") as htps:
            ht_ps = htps.tile([128, 512], f32, name="ht_ps")
            for kt in range(DS // 128):
                nc.tensor.matmul(ht_ps[:, :B], Wb1_sb[:, kt], sfT_sb[:, kt],
                                 start=(kt == 0), stop=(kt == DS // 128 - 1))
            HT_sb = consts.tile([128, B], f16)
            nc.scalar.activation(HT_sb[:], ht_ps[:, :B],
                                 mybir.ActivationFunctionType.Relu,
                                 bias=bias4_sb[:, 3:4], scale=1.0)

        # ---- phase B: bias shard GEMM + AllToAll -----------------------
        a2a_in = dram.tile([B, NSH], f16)
        a2a_out = dram.tile([B, NSH], f16)

        with tc.tile_pool(name="w2", bufs=NT // NG) as w2p, \
             tc.tile_pool(name="bsb", bufs=2) as bsbp, \
             tc.tile_pool(name="bps", bufs=2, space="PSUM") as bpsp:
            w2ts = []
            for g in range(NT // NG):
                w2t = w2p.tile([128, NG, 512], f16, tag="w2t", name=f"w2t{g}")
                nc.sync.dma_start(w2t[:], Wb2s_d[:, ts(g, NG * 512)].rearrange(
                    "p (n w) -> p n w", w=512))
                w2ts.append(w2t)
            for g in range(NT // NG):
                w2t = w2ts[g]
                bsb = bsbp.tile([B, NG, 512], f16, tag="bsb", name=f"bsb{g}")
                for h in range(2):
                    bps = bpsp.tile([B, NG // 2, 512], f32, tag="bps",
                                    name=f"bps{g}_{h}")
                    for n in range(NG // 2):
                        nc.tensor.matmul(bps[:, n], HT_sb[:],
                                         w2t[:, h * (NG // 2) + n],
                                         start=True, stop=True)
                    eng = nc.vector.tensor_copy if h == 0 else nc.scalar.copy
                    eng(bsb[:, ts(h, NG // 2)], bps[:])
                nc.sync.dma_start(
                    a2a_in[:, ts(g, NG * 512)].rearrange("p (n w) -> p n w", w=512),
                    bsb[:])

        nc.gpsimd.collective_compute(
            "AllToAll", mybir.AluOpType.bypass,
            replica_groups=[list(range(N_CORES))],
            ins=[a2a_in.opt()], outs=[a2a_out.opt()],
        )
        # rows of a2a_out: (c2*2 + hi)*4 + s ; cols: l1'*512 + l2
        a2a_v = a2a_out.rearrange("(c2 hi s) (l1 l2) -> hi s l1 c2 l2",
                                  c2=NC1, hi=2, l2=L)

        # ---- phase C: attention (softmax part, no bias needed) ---------
        prj = ctx.enter_context(tc.tile_pool(name="prj", bufs=2))
        vpool = ctx.enter_context(tc.tile_pool(name="vpool", bufs=BPC))
        expp = ctx.enter_context(tc.tile_pool(name="expp", bufs=BPC * NC1))
        smal = ctx.enter_context(tc.tile_pool(name="smal", bufs=BPC * NC1))
        pps = ctx.enter_context(tc.tile_pool(name="pps", bufs=2, space="PSUM"))
        sps = ctx.enter_context(tc.tile_pool(name="sps", bufs=2, space="PSUM"))
        tps = ctx.enter_context(tc.tile_pool(name="tps", bufs=2, space="PSUM"))
        ops = ctx.enter_context(tc.tile_pool(name="ops", bufs=2, space="PSUM"))
        exp_t, rec_t, v_t = {}, {}, {}

        for s in range(BPC):
            q_ps = pps.tile([128, 512], f32, tag="pp", name=f"qps{s}")
            for kt in range(KT):
                nc.tensor.matmul(q_ps[:], wqkv_sb[:, kt, 0], qTin[s][:, kt],
                                 start=(kt == 0), stop=(kt == KT - 1))
            qT_sb = prj.tile([128, L], f16, tag="qT", name=f"qT{s}")
            nc.vector.tensor_scalar_add(qT_sb[:], q_ps[:], bias4_sb[:, 0:1])

            k_ps = pps.tile([128, 512], f32, tag="pp", name=f"kps{s}")
            for kt in range(KT):
                nc.tensor.matmul(k_ps[:], wqkv_sb[:, kt, 1], kTin[s][:, kt],
                                 start=(kt == 0), stop=(kt == KT - 1))
            kT_sb = prj.tile([128, L], f16, tag="kT", name=f"kT{s}")
            nc.vector.tensor_scalar_add(kT_sb[:], k_ps[:], bias4_sb[:, 1:2])

            w_ps = pps.tile([128, 512], f32, tag="pp", name=f"wps{s}")
            for kt in range(KT):
                nc.tensor.matmul(w_ps[:], wqkv_sb[:, kt, 2], vTin[s][:, kt],
                                 start=(kt == 0), stop=(kt == KT - 1))
            vT_sb = prj.tile([128, L], f16, tag="vTs", name=f"vTs{s}")
            nc.vector.tensor_scalar_add(vT_sb[:], w_ps[:], bias4_sb[:, 2:3])
            v_ps = tps.tile([128, 512], f16, tag="tp", name=f"vps{s}",
                            padded_shape=[128, 1024])
            for j in range(NC1):
                nc.tensor.transpose(v_ps[:, ts(j, 128)], vT_sb[:, ts(j, 128)],
                                    ident16)
            v_sb = vpool.tile([128, NC1, DQ], f16, tag="v", name=f"v{s}")
            nc.vector.tensor_copy(v_sb[:], v_ps[:].rearrange("p (j d) -> p j d", j=NC1))
            v_t[s] = v_sb

            for c in range(NC1):
                sc_ps = sps.tile([128, 512], f32, tag="sp", name=f"sc{s}_{c}")
                nc.tensor.matmul(sc_ps[:], qT_sb[:, ts(c, 128)], kT_sb[:],
                                 start=True, stop=True)
                nc.vector.copy_predicated(sc_ps[:], mtile[s][:, c],
                                          c9[:].to_broadcast([128, 512]))
                mx = smal.tile([128, 4], f32, tag="small", name=f"mx{s}_{c}")
                nc.vector.reduce_max(mx[:, 0:1], sc_ps[:],
                                     axis=mybir.AxisListType.X)
                nc.vector.tensor_scalar_mul(mx[:, 1:2], mx[:, 0:1], -SCALE)
                exp_sb = expp.tile([128, L], f16, tag="exp", name=f"exp{s}_{c}")
                nc.scalar.activation(exp_sb[:], sc_ps[:],
                                     mybir.ActivationFunctionType.Exp,
                                     bias=mx[:, 1:2], scale=SCALE,
                                     accum_out=mx[:, 2:3])
                nc.vector.reciprocal(mx[:, 3:4], mx[:, 2:3])
                exp_t[(s, c)] = exp_sb
                rec_t[(s, c)] = mx

        # ---- phase D: post-A2A tail ------------------------------------
        bi = ctx.enter_context(tc.tile_pool(name="bi", bufs=BPC))
        atp = ctx.enter_context(tc.tile_pool(name="atp", bufs=3))
        att = ctx.enter_context(tc.tile_pool(name="att", bufs=2))
        outp = ctx.enter_context(tc.tile_pool(name="outp", bufs=2))

        bias16_t = {}
        for s in range(BPC):
            bias16_t[s] = bi.tile([128, NC1, L], f16, tag="bias16",
                                  name=f"b16_{s}")
            nc.sync.dma_start(bias16_t[s][0:64], a2a_v[0, s])
            nc.sync.dma_start(bias16_t[s][64:128], a2a_v[1, s])
        for s in range(BPC):
            bias16 = bias16_t[s]
            attnT_sb = att.tile([128, NC1, L], f16, tag="attnT", name=f"aT{s}")
            for c in range(NC1):
                biasb = atp.tile([128, L], f16, tag="biasb", name=f"bb{s}_{c}")
                nc.vector.tensor_tensor(biasb[:], bias16[:, c], bb2_sb[:, c],
                                        mybir.AluOpType.add)
                attn = atp.tile([128, L], f16, tag="attn", name=f"at{s}_{c}")
                nc.vector.scalar_tensor_tensor(
                    attn[:], exp_t[(s, c)][:], rec_t[(s, c)][:, 3:4], biasb[:],
                    op0=mybir.AluOpType.mult, op1=mybir.AluOpType.add)
                at_ps = tps.tile([128, 512], f16, tag="tp", name=f"atps{s}_{c}",
                                 padded_shape=[128, 1024])
                for j in range(NC1):
                    nc.tensor.transpose(at_ps[:, ts(j, 128)],
                                        attn[:, ts(j, 128)], ident16)
                nc.scalar.copy(
                    attnT_sb[:, :, ts(c, 128)],
                    at_ps[:].rearrange("p (j d) -> p j d", j=NC1))

            oT_ps = ops.tile([128, 512], f32, tag="op", name=f"oT{s}")
            for j in range(NC1):
                nc.tensor.matmul(oT_ps[:], v_t[s][:, j], attnT_sb[:, j],
                                 start=(j == 0), stop=(j == NC1 - 1))
            oT_sb = outp.tile([128, L], f16, tag="oT", name=f"oTs{s}")
            nc.vector.tensor_copy(oT_sb[:], oT_ps[:])
            o_ps = tps.tile([128, 512], f16, tag="tp", name=f"opss{s}",
                            padded_shape=[128, 1024])
            for c in range(NC1):
                nc.tensor.transpose(o_ps[:, ts(c, 128)], oT_sb[:, ts(c, 128)],
                                    ident16)
            o_sb = outp.tile([128, NC1, DQ], f32, tag="o", name=f"os{s}")
            nc.scalar.copy(o_sb[:], o_ps[:].rearrange("p (c d) -> p c d", c=NC1))
            nc.sync.dma_start(out_d[s].rearrange("(c p) d -> p c d", p=128),
                              o_sb[:])

    nc.compile()
    _cache["nc"] = nc
    return nc


def _prep_in_maps(query, key, value, sf, atten_mask, Wq, bq, Wk, bk, Wv, bv,
                  Wb1, bb1, Wb2, bb2):
    f16 = np.float16
    sfT = np.ascontiguousarray(np.asarray(sf, np.float32).T)
    wqkv = np.ascontiguousarray(
        np.stack([np.asarray(Wq, f16), np.asarray(Wk, f16),
                  np.asarray(Wv, f16)], axis=1))
    bias4 = np.ascontiguousarray(
        np.stack([np.asarray(bq, np.float32), np.asarray(bk, np.float32),
                  np.asarray(bv, np.float32), np.asarray(bb1, np.float32)],
                 axis=1))
    Wb1f = np.ascontiguousarray(np.asarray(Wb1, np.float32))
    bb2r = np.ascontiguousarray(np.asarray(bb2, f16).reshape(L, L))
    Wb2_16 = np.asarray(Wb2, f16)

    in_maps = []
    for i in range(N_CORES):
        sl = slice(BPC * i, BPC * (i + 1))
        in_maps.append({
            "qT": np.ascontiguousarray(
                np.asarray(query[sl], f16).transpose(0, 2, 1)),
            "kT": np.ascontiguousarray(
                np.asarray(key[sl], f16).transpose(0, 2, 1)),
            "vT": np.ascontiguousarray(
                np.asarray(value[sl], f16).transpose(0, 2, 1)),
            "mk": np.ascontiguousarray(np.asarray(atten_mask[sl], np.uint8)),
            "sfT": sfT,
            "wqkv": wqkv,
            "bias4": bias4,
            "Wb1": Wb1f,
            "Wb2s": np.ascontiguousarray(Wb2_16[:, NSH * i: NSH * (i + 1)]),
            "bb2r": bb2r,
        })
    return in_maps


def kernel(**inputs) -> np.ndarray:
    from concourse import bass_utils
    nc = _build()
    in_maps = _prep_in_maps(**inputs)
    res = bass_utils.run_bass_kernel_spmd(
        nc, in_maps, core_ids=list(range(N_CORES)))
    return np.concatenate([r["out"] for r in res.results], axis=0)
